# revision 9
# baseline (speedup 1.0000x reference)
"""DcorLoss kernel — fp8 DoubleRow + symmetric-triangle variant.

Same fp8-DoubleRow psum math as v2 (psum = n_i + n_j - 2 x_i.x_j via one
DR matmul stream; mu^2*I added on diagonal chunks; a = sqrt(psum)), but
exploits symmetry of the distance matrices: only 36 of 64 [128 x 1024]
cells per core are computed, and the 4 diag cells below the block
midline are trimmed to their upper [128 x 512] half (missing transposes
recovered via extra h1 colsums of diag cells 0-3; their h1 pab parts
count twice via a second STT accumulator).

Cell decomposition: 64 row-chunks (I) x 8 col-windows (Jw). Each core c:
  - 8 "diag" cells: I = 8c+ci, window c  (rowsums only; the off-diagonal
    128-chunks inside the diagonal block pair up across cells, so
    counting rowsums once covers them exactly).
  - 28 "upper" cells from a 7-round round-robin tournament of the 8
    blocks: round pairs (m, M), rows from block m, window M; the two
    partner cores take 4 cells each. Counted twice (pab) and both
    rowsums (ACT accum) + colsums (PE f32r ones-matmul into PSUM,
    drained per round) feed the global row sums.

Engine budget per cell: ACT 2x(sqrt+accum) is the wall (~2.4us full
cell; accum drains pipeline under the next ACTIVATE), PE ~2.1us, DVE
~1.4us. DVE accumulate-ops are avoided for bulk work: they run at full
rate (no 4x mode) and sustained DVE saturation trips the HAM half-clock
throttle. Measured ~102us HW exec (vs 286us baseline), rel err 3.4e-4
(gate 2e-2); occasional ~20% slower runs from device power state.
"""

import numpy as np
import ml_dtypes

import concourse.bass as bass
import concourse.tile as tile
from concourse import bacc, mybir
from concourse.bass_utils import run_bass_kernel_spmd

P = 128
N = 8192
NCORES = 8
BLK = 1024
CI_N = 8
W = 1024
JT_N = 8
NK = 128
MU = 16.0
F8 = ml_dtypes.float8_e4m3
NCELL = 36          # 8 diag + 28 upper
NROUND = 7

_programs = {}


def _partner(c, r):
    """Round-robin circle method for 8 teams, rounds 0..6."""
    if c == 7:
        return r
    if r == c:
        return 7
    return (2 * r - c) % 7


def _schedule(c):
    """Per-core cell list: (wx_slot, win_slot, kind, round, k).

    kind: 'diag' or 'upper'. wx_slot: 0..35 into the stationary tile.
    win_slot: 0..7 into the moving tile. Mirrors on host and device.
    """
    cells = []
    for r in range(NROUND):
        for k in range(4):
            cells.append((8 + 4 * r + k, r + 1, "upper", r, k))
    # diag cells last: the kernel tail then has no colsum chain, and the
    # final round's psC drain overlaps the diag phase
    for ci in range(CI_N):
        cells.append((ci, 0, "diag", None, ci))
    return cells


def _wx_chunks(c):
    """Global row-chunk index for each of the 36 stationary slots."""
    chunks = [8 * c + s for s in range(8)]
    for r in range(NROUND):
        p = _partner(c, r)
        m = min(c, p)
        for k in range(4):
            off = k if c == m else 4 + k
            chunks.append(8 * m + off)
    return chunks


def _windows(c):
    """Global window index for each of the 8 moving slots."""
    wins = [c]
    for r in range(NROUND):
        wins.append(max(c, _partner(c, r)))
    return wins


def _build():
    dt = mybir.dt
    f32 = dt.float32
    f32r = dt.float32r
    f8 = dt.float8e4
    A = mybir.AluOpType
    AF = mybir.ActivationFunctionType
    DR = mybir.MatmulPerfMode.DoubleRow

    nc = bacc.Bacc("TRN2", target_bir_lowering=False, debug=False,
                   num_devices=NCORES)

    dWX = nc.dram_tensor("WX", [NK, 2, NCELL * P], f8,
                         kind="ExternalInput").ap()
    dWY = nc.dram_tensor("WY", [NK, 2, NCELL * P], f8,
                         kind="ExternalInput").ap()
    dMX = nc.dram_tensor("MX", [NK, 2, N], f8, kind="ExternalInput").ap()
    dMY = nc.dram_tensor("MY", [NK, 2, N], f8, kind="ExternalInput").ap()
    dEYE = nc.dram_tensor("EYE", [P, P], f8, kind="ExternalInput").ap()
    dEYW = nc.dram_tensor("EYW", [P, 4 * 512], f8, kind="ExternalInput").ap()
    dSEL = nc.dram_tensor("SEL", [P, 16], mybir.dt.float32r,
                          kind="ExternalInput").ap()
    dOUT = nc.dram_tensor("out", [P, 3 * NCELL + 4], f32,
                          kind="ExternalOutput").ap()
    dCS = nc.dram_tensor("cs", [4, (NROUND + 1) * 512], f32,
                         kind="ExternalOutput").ap()

    cells = _schedule(0)   # slot structure is identical on every core

    with tile.TileContext(nc) as tc:
        with tc.tile_pool(name="const", bufs=1) as cp, \
             tc.tile_pool(name="psum", bufs=1, space="PSUM") as pp, \
             tc.tile_pool(name="ab", bufs=4) as abp, \
             tc.tile_pool(name="trd", bufs=2) as trd:

            wx = cp.tile([NK, 2, NCELL * P], f8, tag="wx")
            wy = cp.tile([NK, 2, NCELL * P], f8, tag="wy")
            mx = cp.tile([NK, 2, N], f8, tag="mx")
            my = cp.tile([NK, 2, N], f8, tag="my")
            eye = cp.tile([P, P], f8, tag="eye")
            eyw = cp.tile([P, 4 * 512], f8, tag="eyw")
            # sel[:, 4q+j] = (j == q): ones-selector weights so colsum q
            # lands on psum partition q (matmul base partition must be 0)
            sel = cp.tile([P, 16], f32r, tag="sel")
            colsb = cp.tile([4, (NROUND + 1) * 512], f32, tag="colsb")
            st = [cp.tile([P, NCELL + 4], f32, tag=f"st{q}",
                          name=f"st{q}")
                  for q in range(3)]

            # round-0 cells (slots 8-15, window 1) run first: ship their
            # data first, split across two queues to parallelize the
            # ~0.6us-per-DMA issue latency; diag-phase data (slots 0-7,
            # window 0, eye/eyw) is needed last
            s811 = bass.ds(8 * P, 8 * P)
            sl1 = bass.ts(1, W)
            nc.sync.dma_start(wx[:, :, s811], dWX[:, :, s811])
            nc.scalar.dma_start(wy[:, :, s811], dWY[:, :, s811])
            nc.sync.dma_start(mx[:, :, sl1], dMX[:, :, sl1])
            nc.scalar.dma_start(my[:, :, sl1], dMY[:, :, sl1])
            nc.sync.dma_start(sel[:], dSEL[:])
            for w in range(2, JT_N):
                sl = bass.ts(w, W)
                nc.sync.dma_start(mx[:, :, sl], dMX[:, :, sl])
                nc.sync.dma_start(my[:, :, sl], dMY[:, :, sl])
            srest = bass.ds(16 * P, (NCELL - 16) * P)
            nc.sync.dma_start(wx[:, :, srest], dWX[:, :, srest])
            nc.sync.dma_start(wy[:, :, srest], dWY[:, :, srest])
            s07 = bass.ds(0, 8 * P)
            sl0 = bass.ts(0, W)
            nc.sync.dma_start(wx[:, :, s07], dWX[:, :, s07])
            nc.sync.dma_start(wy[:, :, s07], dWY[:, :, s07])
            nc.sync.dma_start(mx[:, :, sl0], dMX[:, :, sl0])
            nc.sync.dma_start(my[:, :, sl0], dMY[:, :, sl0])
            nc.sync.dma_start(eye[:], dEYE[:])
            nc.sync.dma_start(eyw[:], dEYW[:])

            wz = cp.tile([2, 512], f8, tag="wz")
            nc.vector.memset(wz[:], 0.0)
            wzl = cp.tile([2, P], f8, tag="wzl")
            nc.vector.memset(wzl[:], 0.0)
            for q in range(3):
                wt = pp.tile([P, W], f32, tag="ps", bufs=3)
                for h in range(2):
                    nc.tensor.matmul(wt[:, bass.ts(h, 512)], wzl[:], wz[:],
                                     start=True, stop=True)

            pend = []       # delayed colsum work: (aT, bT, rnd, k)
            pc = {"t": None}

            def emit_colsums():
                if not pend:
                    return
                aT, bT, rnd, k = pend.pop(0)
                if k == 0:
                    pc["t"] = pp.tile([4, 512], f32, tag="pc", bufs=2,
                                      name="pc")
                pct = pc["t"]
                if rnd == NROUND:
                    # diag cells 0-3: colsums of the h1 halves only (a on
                    # psum partition 0, b on partition 1)
                    work = ((aT, 1, 0), (bT, 1, 1))
                else:
                    work = ((aT, 0, 0), (aT, 1, 1), (bT, 0, 2), (bT, 1, 3))
                last_q = work[-1][2]
                for src, h, q in work:
                    nc.tensor.matmul(
                        pct[:, :],
                        sel[:, bass.ts(q, 4)],
                        src[:, bass.ts(h, 512)],
                        start=(k == 0 and q == 0),
                        stop=(k == 3 and q == last_q))
                if k == 3:
                    nc.vector.tensor_copy(
                        colsb[:, bass.ts(rnd, 512)], pct[:, :])

            for idx, (ss, ws, kind, rnd, k) in enumerate(cells):
                psA = pp.tile([P, W], f32, tag="ps", bufs=3)
                psB = pp.tile([P, W], f32, tag="ps", bufs=3)
                diag = kind == "diag"
                ci = k
                trim = diag and ci >= 4      # only cols 512-1023 needed
                hs = (1,) if trim else (0, 1)
                sl = bass.ds(512, 512) if trim else bass.ds(0, W)
                for ps_, wt_, mt_ in ((psA, wx, mx), (psB, wy, my)):
                    for h in hs:
                        nc.tensor.matmul(
                            ps_[:, bass.ts(h, 512)],
                            wt_[:, :, bass.ts(ss, P)],
                            mt_[:, :, bass.ds(ws * W + h * 512, 512)],
                            start=True,
                            stop=not (diag and h == ci // 4),
                            perf_mode=DR)
                    if diag:
                        nc.tensor.matmul(
                            ps_[:, bass.ts(ci // 4, 512)],
                            eye[:], eyw[:, bass.ts(ci % 4, 512)],
                            start=False, stop=True)
                emit_colsums()

                aT = abp.tile([P, W], f32r, tag="ab")
                bT = abp.tile([P, W], f32r, tag="ab")
                nc.scalar.activation(aT[:, sl], psA[:, sl], AF.Sqrt,
                                     accum_out=st[0][:, idx:idx + 1])
                nc.scalar.activation(bT[:, sl], psB[:, sl], AF.Sqrt,
                                     accum_out=st[1][:, idx:idx + 1])
                t0 = trd.tile([P, W], f32, tag="r")
                nc.vector.scalar_tensor_tensor(
                    t0[:, sl], aT[:, sl].bitcast(f32), MU,
                    bT[:, sl].bitcast(f32),
                    op0=A.subtract, op1=A.mult,
                    accum_out=st[2][:, idx:idx + 1])
                if diag and ci < 4:
                    # h1 part counted twice in pab (its transpose in the
                    # trimmed cells is not computed): extra h1-only accum
                    t1 = trd.tile([P, 512], f32, tag="r2", name="t1")
                    h1 = bass.ds(512, 512)
                    nc.vector.scalar_tensor_tensor(
                        t1[:], aT[:, h1].bitcast(f32), MU,
                        bT[:, h1].bitcast(f32),
                        op0=A.subtract, op1=A.mult,
                        accum_out=st[2][:, NCELL + ci:NCELL + ci + 1])
                    pend.append((aT, bT, NROUND, ci))
                if not diag:
                    pend.append((aT, bT, rnd, k))

            while pend:
                emit_colsums()

            nc.sync.dma_start(dOUT[:, 0:NCELL], st[0][:, 0:NCELL])
            nc.sync.dma_start(dOUT[:, NCELL:2 * NCELL],
                              st[1][:, 0:NCELL])
            nc.sync.dma_start(dOUT[:, 2 * NCELL:3 * NCELL + 4],
                              st[2][:, 0:NCELL + 4])
            nc.sync.dma_start(dCS[:], colsb[:])

    nc.compile()
    return nc


def _get_program():
    if "p" not in _programs:
        _programs["p"] = _build()
    return _programs["p"]


def _f8r(a):
    return np.asarray(a, np.float64).astype(F8).astype(np.float64)


def _prep(v):
    v8 = np.asarray(v, np.float32).astype(F8)
    v8d = v8.astype(np.float64)
    assert np.all(np.isfinite(v8d))
    w8d = -2.0 * v8d
    nhat = (v8d * v8d).sum(1)
    hi = _f8r(nhat / 2.0)
    r1 = nhat - 2.0 * hi
    mid = _f8r(r1)
    r2 = r1 - mid
    lo = _f8r(r2)
    nspl = 2.0 * hi + mid + lo
    return dict(v8d=v8d, w8d=w8d, nhat=nhat, hi=hi, mid=mid, lo=lo,
                nspl=nspl)


def _pack_w(pr, row_idx):
    """Stationary tile [NK, 2, NCELL*P] for given global rows."""
    nslot = len(row_idx) // P
    WT = np.zeros((NK, 2, nslot * P), np.float64)
    wb = pr["w8d"][row_idx]
    for t in range(2):
        WT[0:64, t, :] = wb[:, t * 64:(t + 1) * 64].T
    WT[64, 0, :] = pr["hi"][row_idx]
    WT[64, 1, :] = pr["mid"][row_idx]
    WT[65, 0, :] = pr["lo"][row_idx]
    WT[65, 1, :] = 2.0
    WT[66, 0, :] = 1.0
    WT[66, 1, :] = 1.0
    return WT.astype(F8)


def _pack_m(pr, perm):
    MT = np.zeros((NK, 2, N), np.float64)
    vp = pr["v8d"][perm]
    for t in range(2):
        MT[0:64, t, :] = vp[:, t * 64:(t + 1) * 64].T
    MT[64, 0, :] = 2.0
    MT[64, 1, :] = 1.0
    MT[65, 0, :] = 1.0
    MT[65, 1, :] = pr["hi"][perm]
    MT[66, 0, :] = pr["mid"][perm]
    MT[66, 1, :] = pr["lo"][perm]
    return MT.astype(F8)


def make_in_maps(x, y):
    px = _prep(x)
    py = _prep(y)
    eye = (np.eye(P) * MU).astype(F8)
    eyw = np.zeros((P, 4 * 512), np.float64)
    for kk in range(4):
        for p in range(P):
            eyw[p, kk * 512 + kk * P + p] = MU
    eyw = eyw.astype(F8)
    selh = np.zeros((P, 16), np.float32)
    for q in range(4):
        selh[:, 5 * q] = 1.0
    in_maps = []
    for c in range(NCORES):
        chunks = _wx_chunks(c)
        row_idx = np.concatenate(
            [np.arange(I * P, (I + 1) * P) for I in chunks])
        wins = _windows(c)
        perm = np.concatenate(
            [np.arange(wv * W, (wv + 1) * W) for wv in wins])
        in_maps.append({
            "WX": _pack_w(px, row_idx),
            "WY": _pack_w(py, row_idx),
            "MX": _pack_m(px, perm),
            "MY": _pack_m(py, perm),
            "EYE": eye,
            "EYW": eyw,
            "SEL": selh,
        })
    return in_maps, (px, py)


def finalize(results, px, py):
    n = float(N)
    rs_a = np.zeros(N, np.float64)
    rs_b = np.zeros(N, np.float64)
    pab = 0.0
    for c in range(NCORES):
        o = np.asarray(results[c]["out"], np.float64)
        cs = np.asarray(results[c]["cs"], np.float64)
        cells = _schedule(c)
        chunks = _wx_chunks(c)
        wins = _windows(c)
        for idx, (ss, ws, kind, rnd, k) in enumerate(cells):
            I = chunks[ss]
            rows = slice(I * P, (I + 1) * P)
            rs_a[rows] += o[:, idx]
            rs_b[rows] += o[:, NCELL + idx]
            mult = 1.0 if kind == "diag" else 2.0
            pab += mult * o[:, 2 * NCELL + idx].sum()
        # diag 0-3 h1 parts count twice in pab (transpose not computed)
        pab += o[:, 3 * NCELL:3 * NCELL + 4].sum()
        # colsums: round r covers window wins[r+1]
        for r in range(NROUND):
            wv = wins[r + 1]
            seg = cs[:, r * 512:(r + 1) * 512]
            rs_a[wv * W:wv * W + 512] += seg[0]
            rs_a[wv * W + 512:(wv + 1) * W] += seg[1]
            rs_b[wv * W:wv * W + 512] += seg[2]
            rs_b[wv * W + 512:(wv + 1) * W] += seg[3]
        # diag-block patch: trimmed cells 4-7 miss cols [0,512); add the
        # colsums of cells 0-3's h1 halves (their transposes)
        segd = cs[:, NROUND * 512:(NROUND + 1) * 512]
        rs_a[c * W + 512:(c + 1) * W] += segd[0]
        rs_b[c * W + 512:(c + 1) * W] += segd[1]

    def sq_closed(pr):
        sx = pr["v8d"].sum(0)
        Sg = float((sx * sx).sum())
        q_ii = 256.0 + 2.0 * (pr["nspl"] - pr["nhat"])
        return (2.0 * n * pr["nspl"].sum() - 2.0 * Sg + 256.0 * n
                - q_ii.sum())

    sq_a = sq_closed(px)
    sq_b = sq_closed(py)

    sa = rs_a - MU
    sb = rs_b - MU
    sat = sa - n * MU
    sbt = sb - n * MU
    Ua = sat.sum()
    Ub = sbt.sum()
    Sab = pab - MU * (sa.sum() - MU * n * n)
    Saa = sq_a - 2.0 * MU * sa.sum() + MU * MU * n * n
    Sbb = sq_b - 2.0 * MU * sb.sum() + MU * MU * n * n

    sumAB = Sab - 2.0 * np.dot(sat, sbt) / n + Ua * Ub / n ** 2
    sumAA = Saa - 2.0 * np.dot(sat, sat) / n + Ua * Ua / n ** 2
    sumBB = Sbb - 2.0 * np.dot(sbt, sbt) / n + Ub * Ub / n ** 2

    inv_n2 = 1.0 / (n * n)
    dcor = (-np.sqrt(sumAB * inv_n2)
            / np.sqrt(np.sqrt(sumAA * inv_n2) * np.sqrt(sumBB * inv_n2)))
    return np.asarray(dcor, dtype=np.float32)


def run(x, y, mm_mode=None, trace=False, tmpdir=None):
    nc = _get_program()
    in_maps, (px, py) = make_in_maps(x, y)
    res = run_bass_kernel_spmd(nc, in_maps, core_ids=list(range(NCORES)),
                               trace=trace, tmpdir=tmpdir)
    return finalize(res.results, px, py), res


def kernel(x, y):
    val, _ = run(x, y)
    return val
